# revision 8
# baseline (speedup 1.0000x reference)
"""Gumbel-softmax sparse attention on 8 TRN2 NeuronCores.

reference math (f32):
    scores = Q @ K.T / sqrt(d_k)          # (n_genes, z_dim)
    p_attn = softmax(scores + gumbel(key=42))
    recon  = p_attn @ V                   # (n_genes, n_cells)
    return recon, p_attn

Sharding: rows (n_genes) of Q/g/outputs split across 8 cores; K and V
replicated. Device kernel per core:
    MM1 (f32r):  S = qT.T @ kT            (qT pre-scaled by 1/sqrt(d_k) on host)
    softmax:     max-subtracted, gumbel noise g added on DVE, exp on ACT with
                 fused row-sum, scale by 1/rowsum
    transpose:   PE-transpose p tiles (128x128) so z lands on partitions
    MM2 (bf16):  recon = pT.T @ V, accumulated in PSUM f32, stored bf16

Host side: transposes/scales Q, transposes K, casts V to bf16, computes the
gumbel noise with jax on CPU (bit-identical to the reference), and casts the
bf16 recon shards back to f32.
"""

import numpy as np

N_GENES, Z_DIM, D_K, N_CELLS = 16384, 1024, 128, 4096
N_CORES = 8
M_SHARD = N_GENES // N_CORES  # 2048
P = 128


def build_nc(m_shard=M_SHARD, z_dim=Z_DIM, d_k=D_K, n_cells=N_CELLS, mm1_f32r=True):
    import concourse.bass as bass
    import concourse.mybir as mybir
    from concourse import bacc, masks, tile
    from contextlib import ExitStack

    f32 = mybir.dt.float32
    f32r = mybir.dt.float32r
    bf16 = mybir.dt.bfloat16
    Alu = mybir.AluOpType
    Act = mybir.ActivationFunctionType

    n_m = m_shard // P          # m tiles (16)
    zh = z_dim // 2             # 512
    NZ = z_dim // P             # 8
    CT = 512
    n_c = n_cells // CT         # 8

    mm1_dt = f32r if mm1_f32r else f32
    nc = bacc.Bacc(None, target_bir_lowering=False)
    qT_d = nc.declare_dram_parameter("qT", [d_k, m_shard], mm1_dt, isOutput=False)
    kT_d = nc.declare_dram_parameter("kT", [d_k, z_dim], mm1_dt, isOutput=False)
    v_d = nc.declare_dram_parameter("v", [z_dim, n_cells], bf16, isOutput=False)
    g_d = nc.declare_dram_parameter("g", [m_shard, z_dim], f32, isOutput=False)
    recon_d = nc.declare_dram_parameter("recon", [m_shard, n_cells], bf16, isOutput=True)
    p_d = nc.declare_dram_parameter("p_attn", [m_shard, z_dim], f32, isOutput=True)

    with tile.TileContext(nc) as tc, ExitStack() as ctx:
        const = ctx.enter_context(tc.tile_pool(name="const", bufs=1))
        gpool = ctx.enter_context(tc.tile_pool(name="gpool", bufs=3))
        sapool = ctx.enter_context(tc.tile_pool(name="sapool", bufs=2))
        epool = ctx.enter_context(tc.tile_pool(name="epool", bufs=2))
        ppool = ctx.enter_context(tc.tile_pool(name="ppool", bufs=3))
        ptpool = ctx.enter_context(tc.tile_pool(name="ptpool", bufs=4))
        rspool = ctx.enter_context(tc.tile_pool(name="rspool", bufs=2))
        stat = ctx.enter_context(tc.tile_pool(name="stat", bufs=8))
        spsum = ctx.enter_context(tc.tile_pool(name="spsum", bufs=2, space="PSUM"))
        ptpsum = ctx.enter_context(tc.tile_pool(name="ptpsum", bufs=2, space="PSUM"))
        rpsum = ctx.enter_context(tc.tile_pool(name="rpsum", bufs=3, space="PSUM"))

        ident = const.tile([P, P], f32)
        masks.make_identity(nc, ident[:])
        kT_sb = const.tile([P, z_dim], mm1_dt)
        nc.sync.dma_start(kT_sb[:], kT_d[:, :])
        qT_sb = const.tile([P, m_shard], mm1_dt)
        nc.sync.dma_start(qT_sb[:], qT_d[:, :])
        v_sb = const.tile([P, NZ * n_cells], bf16)
        # Chunk the V load by c-column group (c outer, j inner) so MM2 of the
        # first m-tile unblocks per c-group after ~1MB instead of waiting for
        # the full 8MB replica — cuts the prologue PE bubble.
        for c in range(n_c):
            for j in range(NZ):
                nc.sync.dma_start(
                    v_sb[:, j * n_cells + c * CT : j * n_cells + (c + 1) * CT],
                    v_d[j * P : (j + 1) * P, c * CT : (c + 1) * CT],
                )

        def stage_A(i):
            """DMA gumbel tile; MM1 into two PSUM halves (f32r)."""
            g_sb = gpool.tile([P, z_dim], f32, tag="g")
            nc.sync.dma_start(g_sb[:], g_d[i * P : (i + 1) * P, :])
            s0 = spsum.tile([P, zh], f32, tag="s")
            s1 = spsum.tile([P, zh], f32, tag="s")
            q_sl = qT_sb[:, i * P : (i + 1) * P]
            nc.tensor.matmul(s0[:], q_sl, kT_sb[:, 0:zh],
                             start=True, stop=True)
            nc.tensor.matmul(s1[:], q_sl, kT_sb[:, zh:z_dim],
                             start=True, stop=True)
            return g_sb, s0, s1

        def stage_B(i, g_sb, s0, s1):
            """softmax(S + g) along free dim -> p_sb (f32), DMA p out.

            No max-subtraction: logits = s + g stay below ~25, exp is safe
            in f32. (tensor_tensor_reduce and bias-AP activation both fault
            on this runtime, so the plain-op formulation is used.)"""
            sadd = sapool.tile([P, z_dim], f32, tag="sadd")
            nc.vector.tensor_add(sadd[:, 0:zh], s0[:], g_sb[:, 0:zh])
            nc.vector.tensor_add(sadd[:, zh:z_dim], s1[:], g_sb[:, zh:z_dim])
            e = epool.tile([P, z_dim], f32, tag="e")
            rs0 = stat.tile([P, 1], f32, tag="st")
            rs1 = stat.tile([P, 1], f32, tag="st")
            nc.scalar.activation(e[:, 0:zh], sadd[:, 0:zh], Act.Exp,
                                 accum_out=rs0[:])
            nc.scalar.activation(e[:, zh:z_dim], sadd[:, zh:z_dim], Act.Exp,
                                 accum_out=rs1[:])
            recip = stat.tile([P, 1], f32, tag="st")
            nc.vector.tensor_add(recip[:], rs0[:], rs1[:])
            nc.vector.reciprocal(recip[:], recip[:])
            p_sb = ppool.tile([P, z_dim], f32, tag="p")
            nc.vector.tensor_scalar_mul(p_sb[:], e[:], recip[:])
            nc.sync.dma_start(p_d[i * P : (i + 1) * P, :], p_sb[:])
            return p_sb

        def stage_D(i, p_sb):
            """PE-transpose p into z-on-partition layout, cast to bf16."""
            pts = []
            for h in range(2):
                ptp = ptpsum.tile([P, zh], f32, tag="ptp")
                for jj in range(4):
                    blk = h * 4 + jj
                    nc.tensor.transpose(
                        ptp[:, jj * P : (jj + 1) * P],
                        p_sb[:, blk * P : (blk + 1) * P],
                        ident[:])
                ptsb = ptpool.tile([P, zh], bf16, tag="pt")
                nc.scalar.copy(ptsb[:], ptp[:])
                pts.append(ptsb)
            return pts

        def stage_E(i, pts):
            """MM2: recon tile = pT.T @ V (bf16), PSUM accum, store bf16."""
            rsb = rspool.tile([P, n_cells], bf16, tag="rsb")
            for c in range(n_c):
                rp = rpsum.tile([P, CT], f32, tag="r")
                for j in range(NZ):
                    pt = pts[j // 4]
                    lhs = pt[:, (j % 4) * P : (j % 4 + 1) * P]
                    rhs = v_sb[:, j * n_cells + c * CT : j * n_cells + c * CT + CT]
                    nc.tensor.matmul(rp[:], lhs, rhs,
                                     start=(j == 0), stop=(j == NZ - 1))
                nc.scalar.copy(rsb[:, c * CT : (c + 1) * CT], rp[:])
            nc.sync.dma_start(recon_d[i * P : (i + 1) * P, :], rsb[:])

        # software pipeline: PE order per iter = [D(i+1), E(i)]; softmax of
        # tile i+2 and MM1 of tile i+3 run on ACT/DVE/PE-gaps during E(i).
        A = {}
        Bv = {}
        D = {}
        A[0] = stage_A(0)
        Bv[0] = stage_B(0, *A[0])
        if n_m > 1:
            A[1] = stage_A(1)
        D[0] = stage_D(0, Bv[0])
        if n_m > 1:
            Bv[1] = stage_B(1, *A[1])
        if n_m > 2:
            A[2] = stage_A(2)
        for i in range(n_m):
            if i + 1 < n_m:
                D[i + 1] = stage_D(i + 1, Bv[i + 1])
            stage_E(i, D[i])
            if i + 2 < n_m:
                Bv[i + 2] = stage_B(i + 2, *A[i + 2])
            if i + 3 < n_m:
                A[i + 3] = stage_A(i + 3)

    nc.compile()
    return nc


def _gumbel_host(shape):
    import jax
    import jax.numpy as jnp

    cpu = jax.devices("cpu")[0]
    with jax.default_device(cpu):
        g = jax.random.gumbel(jax.random.key(42), shape, jnp.float32)
        return np.asarray(g)


def kernel(query, key, value):
    import os
    os.environ.setdefault("NEURON_RT_RESET_CORES", "1")
    import ml_dtypes
    from concourse.bass_utils import run_bass_kernel_spmd

    query = np.asarray(query, dtype=np.float32)
    key = np.asarray(key, dtype=np.float32)
    value = np.asarray(value, dtype=np.float32)

    qT = np.ascontiguousarray(query.T) * np.float32(1.0 / np.sqrt(D_K))
    kT = np.ascontiguousarray(key.T)
    v_bf = value.astype(ml_dtypes.bfloat16)
    g = _gumbel_host((N_GENES, Z_DIM))

    nc = build_nc()
    in_maps = []
    for c in range(N_CORES):
        sl = slice(c * M_SHARD, (c + 1) * M_SHARD)
        in_maps.append({
            "qT": np.ascontiguousarray(qT[:, sl]),
            "kT": kT,
            "v": v_bf,
            "g": np.ascontiguousarray(g[sl, :]),
        })
    res = run_bass_kernel_spmd(nc, in_maps, core_ids=list(range(N_CORES)))
    recon = np.concatenate(
        [r["recon"].astype(np.float32) for r in res.results], axis=0)
    p_attn = np.concatenate([r["p_attn"] for r in res.results], axis=0)
    return recon, p_attn
